# revision 1
# baseline (speedup 1.0000x reference)
"""Trainium2 Bass kernel for nn_CubeSimulator.

Reference computation: a 128^3 spatial grid is rotated (Rz(sky_rot) then
Rx(inclination)), a rotation-curve velocity field and an exponential-disk
intensity field are evaluated, an 80-channel Gaussian KDE over the
line-of-sight velocity reduces the third grid axis, and the [80,128,128]
cube is avg-pooled (5,4,4) to [16,32,32].

Kernel strategy
---------------
* Exact point-reflection symmetry: (i,j,k) -> (127-i,127-j,127-k) negates the
  rotated coordinates, so vz -> -vz and src is unchanged, giving
  cube[v, 127-i, 127-j] == cube[79-v, i, j] (the grid linspace is exactly
  antisymmetric in fp32).  Only the i < 64 half is computed on-device; the
  host mirrors the pooled output.  2x savings on everything.
* Sharding: the 64 computed sky-plane rows are split 8 rows/core over the 8
  NeuronCores (data-parallel over image rows, per the sharding hint).
* On-device layout: partitions = k (the reduced grid axis), free dims =
  (i_local=8) x (j=128) = 1024.  Per velocity channel the KDE summand is
  exp(L - (z_v - vz)^2/sig^2)  (intensity folded into the exponent), expanded
  as exp(a*z_v + b + c_v) with per-point a = 2 vz/sig^2,
  b = L - vz^2/sig^2 and per-channel c_v = -z_v^2/sig^2.  Inner loop:
    - one DVE scalar_tensor_tensor:  ARG = a*z_v + b
    - one ACT Exp (bias=c_v), emitting bf16 weights
    - PE matmuls against a ones-vector: sum over k (partitions) with
      channel-group accumulation in PSUM (the velocity avg-pool for free)
  Channels are processed in symmetric pairs (v, 79-v) which share c_v, so
  one ACT instruction covers both (large-N instructions amortize the ACT
  fixed overhead).
* All input-dependent scalars enter as DRAM tensors (per-partition operand
  columns), so the compiled program is input-independent and cached.
"""

import sys

for _p in ("/opt/trn_rl_repo",):
    if _p not in sys.path:
        sys.path.insert(0, _p)

import numpy as np
import ml_dtypes

# ---------------- problem constants (compile-time, model-intrinsic) --------
IMAGE_RES = 128          # internal spatial resolution
VEL_RES = 80             # internal velocity channels
VEL_UP = 5
IMG_UP = 4
N_CORES = 8
HALF_I = IMAGE_RES // 2          # 64 computed rows
ROWS_PER_CORE = HALF_I // N_CORES  # 8
FREE = ROWS_PER_CORE * IMAGE_RES   # 1024 free elements per partition
CUBE_FOV = 1000.0
M_TO_PC = 1.0 / 3.086e16
V_MAX_PC = np.float32(200000.0 * M_TO_PC)
R_C = np.float32(0.1 * CUBE_FOV)
R_D = np.float32(0.3 * CUBE_FOV)
H_Z = np.float32(0.05 * CUBE_FOV)
VEL_MIN = -300000.0
VEL_MAX = 300000.0

_INV_RD2 = 1.0 / (float(R_D) * float(R_D))  # Sqrt scale -> r2d/R_D
_EPS_R2D2 = np.float32(1e-25)  # host-folded guard for the reciprocal

# scalar-column layout inside the packed small input `sm`
# sm = [ nsz(1) | ciz(1) | zv2(80) | cv(40) | scal(8) ]
_C_NSZ = 0
_C_CIZ = 1
_C_ZV2 = 2                        # per-channel 2*z_v/sig^2
_C_CV = _C_ZV2 + VEL_RES          # 82
_C_SCAL = _C_CV + VEL_RES // 2    # 122
SM_COLS = _C_SCAL + 8             # 130
# scal sub-columns
_S_NSIG = _C_SCAL + 1    # -1/sig^2
_S_RC2 = _C_SCAL + 3     # R_C^2
_S_NEGH = _C_SCAL + 4    # -1/(2 H_Z^2)

_EARLY_SPLIT_PAIRS = 4   # pairs whose ACT op is halved to bridge startup

_CACHE = {}


def _build_program():
    from concourse import bacc, mybir, tile

    f32 = mybir.dt.float32
    bf16 = mybir.dt.bfloat16
    AF = mybir.ActivationFunctionType
    OP = mybir.AluOpType

    nc = bacc.Bacc(None)

    pk_d = nc.dram_tensor("pk", [128, 4 * FREE], f32, kind="ExternalInput")
    sm_d = nc.dram_tensor("sm", [128, SM_COLS], f32, kind="ExternalInput")
    ones_d = nc.dram_tensor("ones", [128, 64], bf16, kind="ExternalInput")
    out_d = nc.dram_tensor("out", [16, 1024], f32, kind="ExternalOutput")

    with tile.TileContext(nc) as tc:
        with (
            tc.tile_pool(name="inp", bufs=1) as inp,
            tc.tile_pool(name="fld", bufs=1) as fld,
            tc.tile_pool(name="arg", bufs=6) as argp,
            tc.tile_pool(name="wp", bufs=6) as wp,
            tc.tile_pool(name="psum", bufs=6, space="PSUM") as psum,
            tc.tile_pool(name="ob", bufs=4) as obp,
        ):
            pk = inp.tile([128, 4 * FREE], f32)
            sm = inp.tile([128, SM_COLS], f32)
            ones = inp.tile([128, 64], bf16)
            # small inputs ride the gpsimd SWDGE queue so the big pk
            # chunks start immediately on the sync queue
            nc.gpsimd.dma_start(sm[:], sm_d[:])
            nc.gpsimd.dma_start(ones[:], ones_d[:])
            # 256KB chunks ordered by when the field chains need them:
            # pa/pb/prx2 halves feed the chain heads, pc only at vzt
            H2 = FREE // 2
            for _c in (0, 2, 4, 1, 3, 5, 6, 7):
                nc.sync.dma_start(pk[:, _c * H2:(_c + 1) * H2],
                                  pk_d[:, _c * H2:(_c + 1) * H2])

            pa = pk[:, 0 * FREE:1 * FREE]
            pb = pk[:, 1 * FREE:2 * FREE]
            prx2 = pk[:, 2 * FREE:3 * FREE]
            pc = pk[:, 3 * FREE:4 * FREE]

            def col(i):
                return sm[:, i:i + 1]

            # ---- field: vz and b = L - vz^2/sig^2, in two 512 halves so the
            # KDE pipeline can start on half 0 while half 1 is in flight ----
            y2 = fld.tile([128, FREE], f32)
            r2d2 = fld.tile([128, FREE], f32)
            z2 = fld.tile([128, FREE], f32)
            q = fld.tile([128, FREE], f32)
            den = fld.tile([128, FREE], f32)
            rec = fld.tile([128, FREE], f32)
            u = fld.tile([128, FREE], f32)
            su = fld.tile([128, FREE], f32)
            vzt = fld.tile([128, FREE], f32)
            nvs = fld.tile([128, FREE], f32)
            slq = fld.tile([128, FREE], f32)
            t2 = fld.tile([128, FREE], f32)
            bb = fld.tile([128, FREE], f32)
            scratch = fld.tile([128, FREE], f32)
            qc = fld.tile([128, FREE], f32)

            V = nc.vector
            G = nc.gpsimd
            HALF = FREE // 2
            # half 0 on DVE, half 1 on gpsimd: the two chains run in
            # parallel, halving time-to-first-Exp (reciprocal is DVE-only).
            # Emission staged so both halves' reciprocals get early DVE
            # priority.
            CHUNKS = ((0, V), (1, G))

            def sl_of(h):
                return slice(h * HALF, (h + 1) * HALF)

            roty = fld.tile([128, FREE], f32)
            rotz = fld.tile([128, FREE], f32)
            for c, E in CHUNKS:
                s = sl_of(c)
                # rot_y = A + (-si*z_k); rot_z = B + (ci*z_k)
                E.tensor_scalar_add(roty[:, s], pa[:, s], col(_C_NSZ))
                E.tensor_scalar_add(rotz[:, s], pb[:, s], col(_C_CIZ))
                E.tensor_mul(y2[:, s], roty[:, s], roty[:, s])
                E.tensor_add(r2d2[:, s], y2[:, s], prx2[:, s])  # rx2 has +eps
                E.tensor_mul(z2[:, s], rotz[:, s], rotz[:, s])
                E.tensor_add(q[:, s], r2d2[:, s], z2[:, s])
                # den = (q + R_C^2) * r2d2  (Pool stt only supports
                # (mult, add); decompose on the gpsimd chunks)
                if E is V:
                    E.scalar_tensor_tensor(den[:, s], q[:, s], col(_S_RC2),
                                           r2d2[:, s], op0=OP.add, op1=OP.mult)
                else:
                    E.tensor_scalar_add(qc[:, s], q[:, s], col(_S_RC2))
                    E.tensor_mul(den[:, s], qc[:, s], r2d2[:, s])
            for c, _E in CHUNKS:
                s = sl_of(c)
                V.reciprocal_approx_accurate(rec[:, s], den[:, s],
                                             scratch[:, s])
            for c, E in CHUNKS:
                s = sl_of(c)
                E.tensor_mul(u[:, s], q[:, s], rec[:, s])
                nc.scalar.activation(su[:, s], u[:, s], AF.Sqrt)
                E.tensor_mul(vzt[:, s], su[:, s], pc[:, s])
                # nvs = (vz * -1/sig^2) * vz
                if E is V:
                    E.scalar_tensor_tensor(nvs[:, s], vzt[:, s], col(_S_NSIG),
                                           vzt[:, s], op0=OP.mult, op1=OP.mult)
                else:
                    E.tensor_scalar_mul(qc[:, s], vzt[:, s], col(_S_NSIG))
                    E.tensor_mul(nvs[:, s], qc[:, s], vzt[:, s])
                # slq = sqrt(r2d2 / R_D^2)
                nc.scalar.activation(slq[:, s], r2d2[:, s], AF.Sqrt,
                                     scale=_INV_RD2)
                # t2 = z2 * (-1/(2 H_Z^2)) + nvs ;  b = t2 - slq
                if E is V:
                    E.scalar_tensor_tensor(t2[:, s], z2[:, s], col(_S_NEGH),
                                           nvs[:, s], op0=OP.mult, op1=OP.add)
                else:
                    E.tensor_scalar_mul(qc[:, s], z2[:, s], col(_S_NEGH))
                    E.tensor_add(t2[:, s], qc[:, s], nvs[:, s])
                E.tensor_sub(bb[:, s], t2[:, s], slq[:, s])

            # ---- KDE: symmetric channel pairs (v, 79-v) ----
            psum_tiles = {}
            grp_count = {}
            # Channel pairs (v, 79-v) have exactly opposite z_v (the host
            # forces zv2 antisymmetric), so arg_{79-v} = 2*b - arg_v
            # (2*b is exact in fp32).  Per-pair engine configs, greedily
            # balanced (cost-model ns):
            #   stt: both channels via stt on DVE
            #   mix: arg_v stt on DVE, arg_{79-v} = bb2 - arg_v on gpsimd
            #   gp:  m = vzt*zv2 ; arg_v = m + b ; arg_{79-v} = bb2 - arg_v
            bb2 = fld.tile([128, FREE], f32)
            for c, E in CHUNKS:
                s = sl_of(c)
                E.tensor_add(bb2[:, s], bb[:, s], bb[:, s])
            eng_t = {"dve": 20_400.0, "gp": 7_900.0}
            CFG = [(2224, 0, "stt"), (1112, 853, "mix"), (0, 2559, "gp")]
            for v in range(VEL_RES // 2):
                vm = VEL_RES - 1 - v
                split = v < _EARLY_SPLIT_PAIRS or v == VEL_RES // 2 - 1
                arg = argp.tile([128, 2 * FREE], f32, tag="arg")
                if split:
                    # gpsimd is still busy with the half-1 field chain at
                    # startup; keep the early pairs entirely on DVE
                    best = CFG[0]
                else:
                    best = min(CFG, key=lambda c: max(eng_t["dve"] + c[0],
                                                      eng_t["gp"] + c[1]))
                eng_t["dve"] += best[0]
                eng_t["gp"] += best[1]
                mode = best[2]
                mt = None
                if mode != "stt":
                    mt = argp.tile([128, FREE], f32, tag="mt", bufs=2)
                w = wp.tile([128, 2 * FREE], bf16, tag="w")

                def emit_args(fs, asl_v, asl_m):
                    """fs: field slice; asl_v/asl_m: arg slices for v, 79-v"""
                    if mode == "stt":
                        V.scalar_tensor_tensor(
                            arg[:, asl_v], vzt[:, fs], col(_C_ZV2 + v),
                            bb[:, fs], op0=OP.mult, op1=OP.add)
                        V.scalar_tensor_tensor(
                            arg[:, asl_m], vzt[:, fs], col(_C_ZV2 + vm),
                            bb[:, fs], op0=OP.mult, op1=OP.add)
                    elif mode == "mix":
                        V.scalar_tensor_tensor(
                            arg[:, asl_v], vzt[:, fs], col(_C_ZV2 + v),
                            bb[:, fs], op0=OP.mult, op1=OP.add)
                        G.tensor_sub(arg[:, asl_m], bb2[:, fs], arg[:, asl_v])
                    else:
                        G.tensor_scalar_mul(mt[:, fs], vzt[:, fs],
                                            col(_C_ZV2 + v))
                        G.tensor_add(arg[:, asl_v], mt[:, fs], bb[:, fs])
                        G.tensor_sub(arg[:, asl_m], bb2[:, fs], arg[:, asl_v])

                if split:
                    # layout [ch0h0|ch1h0|ch0h1|ch1h1]: Exp on half 0 can run
                    # before the field finishes half 1
                    for hq in range(2):
                        fs = sl_of(hq)
                        emit_args(fs,
                                  slice(2 * hq * HALF, (2 * hq + 1) * HALF),
                                  slice((2 * hq + 1) * HALF,
                                        (2 * hq + 2) * HALF))
                        nc.scalar.activation(
                            w[:, 2 * hq * HALF:2 * (hq + 1) * HALF],
                            arg[:, 2 * hq * HALF:2 * (hq + 1) * HALF],
                            AF.Exp, bias=col(_C_CV + v))
                else:
                    emit_args(slice(0, FREE), slice(0, FREE),
                              slice(FREE, 2 * FREE))
                    nc.scalar.activation(w[:], arg[:], AF.Exp,
                                         bias=col(_C_CV + v))

                for hh, ch in enumerate((v, vm)):
                    vo = ch // VEL_UP
                    if vo not in psum_tiles:
                        # one bank; halves land on partition rows 0 and 64
                        # so the PSUM->SBUF copy reads 512/partition, not
                        # 1024 (matmul out base must be 0/32/64)
                        psum_tiles[vo] = psum.tile([128, HALF], f32,
                                                   tag="acc", name=f"acc{vo}")
                        grp_count[vo] = 0
                    pt = psum_tiles[vo]
                    cnt = grp_count[vo]
                    if split:
                        mm = [((2 * ck + hh) * HALF, (2 * ck + hh + 1) * HALF,
                               ck, 0, HALF) for ck in range(2)]
                    else:
                        mm = [(hh * FREE + ck * HALF,
                               hh * FREE + (ck + 1) * HALF, ck, 0, HALF)
                              for ck in range(2)]
                    for w0, w1, rb, o0, o1 in mm:
                        nc.tensor.matmul(
                            pt[64 * rb:64 * rb + 64, o0:o1], ones[:, :],
                            w[:, w0:w1],
                            start=(cnt == 0), stop=(cnt == VEL_UP - 1),
                            # rows 0-63 and 64-127 are separate groups on HW;
                            # CoreSim's zero-region check ignores the
                            # partition base and false-positives
                            skip_group_check=True,
                        )
                    grp_count[vo] = cnt + 1
                    if grp_count[vo] == VEL_UP:
                        # v-pooled cube rows; (i,j) spatial pooling + scaling
                        # happens on the host.  DMA cannot read PSUM and
                        # compute APs need partition step 1, so copy the
                        # contiguous [65, 512] block (cost ~ free size) and
                        # let the DMA pick rows 0 and 64.  The very last
                        # completion copies via the then-idle ACT so the two
                        # final copies run in parallel.
                        ot = obp.tile([65, HALF], f32, tag="ob",
                                      name=f"ot{vo}")
                        if v == VEL_RES // 2 - 1 and hh == 1:
                            nc.scalar.activation(ot[:, :], pt[0:65, :],
                                                 AF.Copy)
                        else:
                            V.tensor_copy(ot[:, :], pt[0:65, :])
                        nc.sync.dma_start(
                            out_d[vo, :].rearrange("(q n) -> q n", q=2),
                            ot[0:65:64, :])
                        del psum_tiles[vo]

    nc.finalize()  # Bacc: runs compile() passes (wait splitting, reg alloc)
    return nc


def _host_inputs(inclination, sky_rot, line_broadening):
    f32 = np.float32
    inc = f32(inclination)
    rot = f32(sky_rot)
    lb = f32(line_broadening)
    ci, si = f32(np.cos(inc)), f32(np.sin(inc))
    cr, sr = f32(np.cos(rot)), f32(np.sin(rot))
    sig_sq = f32(lb * lb)

    lin = np.linspace(-CUBE_FOV, CUBE_FOV, IMAGE_RES, dtype=f32)
    z_labels = np.linspace(f32(VEL_MIN * M_TO_PC), f32(VEL_MAX * M_TO_PC),
                           VEL_RES, dtype=f32)

    sm = np.zeros((128, SM_COLS), dtype=f32)
    sm[:, _C_NSZ] = (-si * lin).astype(f32)          # -si * z_k
    sm[:, _C_CIZ] = (ci * lin).astype(f32)           # ci * z_k
    # 2*z_v/sig^2, matching fp32 eval order z_v * (2/sig^2); forced exactly
    # antisymmetric (z_labels is antisymmetric to 1 ulp) so the device can
    # compute arg_{79-v} = b - m from m = vzt*zv2_v
    zv2 = (z_labels * f32(2.0 / sig_sq)).astype(f32)
    zv2[VEL_RES // 2:] = -zv2[:VEL_RES // 2][::-1]
    sm[:, _C_ZV2:_C_ZV2 + VEL_RES] = zv2
    cvv = (-(z_labels[:40] * z_labels[:40]) / sig_sq).astype(f32)
    sm[:, _C_CV:_C_CV + 40] = cvv
    sm[:, _S_NSIG] = f32(-1.0 / sig_sq)
    sm[:, _S_RC2] = f32(float(R_C) * float(R_C))
    sm[:, _S_NEGH] = f32(-1.0 / (2.0 * float(H_Z) * float(H_Z)))
    ones = np.ones((128, 64), dtype=ml_dtypes.bfloat16)

    in_maps = []
    for c in range(N_CORES):
        x = lin[8 * c: 8 * c + 8][:, None]                 # [8,1]
        y = lin[None, :]                                   # [1,128]
        y1 = (sr * x + cr * y).astype(f32)
        A = (ci * y1).astype(f32).reshape(-1)
        B = (si * y1).astype(f32).reshape(-1)
        rot_x = (cr * x - sr * y).astype(f32)
        rx2 = (rot_x * rot_x + _EPS_R2D2).astype(f32).reshape(-1)
        C = (-si * V_MAX_PC * rot_x).astype(f32).reshape(-1)
        pkrow = np.concatenate([A, B, rx2, C]).astype(f32)  # [4*FREE]
        pk = np.ascontiguousarray(np.broadcast_to(pkrow, (128, 4 * FREE)))
        in_maps.append({"pk": pk, "sm": sm, "ones": ones})
    return in_maps


def _run(in_maps, trace=False, **kwargs):
    from concourse.bass_utils import run_bass_kernel_spmd
    if "nc" not in _CACHE:
        _CACHE["nc"] = _build_program()
    return run_bass_kernel_spmd(_CACHE["nc"], in_maps,
                                list(range(N_CORES)), trace=trace, **kwargs)


def _assemble(results, line_broadening):
    f32 = np.float32
    lb = f32(line_broadening)
    sig_sq = f32(lb * lb)
    pref = f32(1.0 / np.sqrt(2.0 * np.pi * sig_sq))
    scale = f32(pref / f32(VEL_UP * IMG_UP * IMG_UP))
    parts = []
    for r in results:
        cube = np.asarray(r["out"]).reshape(16, 2, 4, 32, 4)  # vo,io,di,jo,dj
        pooled = cube.sum(axis=(2, 4), dtype=np.float32) * scale  # [16,2,32]
        parts.append(pooled.astype(f32))
    half = np.concatenate(parts, axis=1)
    full = np.empty((16, 32, 32), dtype=np.float32)
    full[:, :16, :] = half
    full[:, 16:, :] = half[::-1, ::-1, ::-1]
    return full


def kernel(inclination, sky_rot, line_broadening):
    in_maps = _host_inputs(inclination, sky_rot, line_broadening)
    res = _run(in_maps)
    return _assemble(res.results, line_broadening)



# revision 11
# speedup vs baseline: 2.9626x; 2.9626x over previous
"""Trainium2 Bass kernel for nn_CubeSimulator.

Reference computation: a 128^3 spatial grid is rotated (Rz(sky_rot) then
Rx(inclination)), a rotation-curve velocity field and an exponential-disk
intensity field are evaluated, an 80-channel Gaussian KDE over the
line-of-sight velocity reduces the third grid axis, and the [80,128,128]
cube is avg-pooled (5,4,4) to [16,32,32].

Kernel strategy
---------------
* Point-reflection symmetry (as in the earlier exp-based kernel): only the
  i < 64 half of the sky plane is computed; the host mirrors the pooled
  output (cube[vo, 127-i, 127-j] == cube[15-vo, i, j]).
* erf-difference identity: the output only needs GROUPS of 5 adjacent
  velocity channels summed (the avg-pool).  The channel spacing is
  0.38 sigma, so the sum of the 5 Gaussians over a group equals
  (sqrt(pi)/2beta) * [erf(beta(u-5t+.5)) - erf(beta(u-5(t+1)+.5))]
  to ~0.6% of the group max (Euler-Maclaurin edge error), where
  u = (vz - z_0)/Delta is the velocity in channel units and t indexes
  GROUP EDGES.  Adjacent groups share edges, so 17 edge evaluations
  replace 80 exponentials - and the saturated edges (|arg| large for
  every point of a shard) are free (+-1 exactly).
* erf itself is evaluated as tanh(a*y + b*y^3) (max fit error 2.8e-4);
  tanh shares the ACT 'exp_and_others' table with exp, so the whole
  program needs only sqrt -> exp_and_others, one table switch.
* Column sharding: the 8192 computed (i,j) columns are sorted by their
  active-edge interval and packed into 32 groups of 256 columns; each
  core gets 4 slot-groups with per-slot edge windows (SPMD instruction
  count = sum of per-slot-class maximum widths, ~25 edge-slots of 256
  columns vs 13 full-width edges).  Edge positions/coefficients are
  per-core DATA (sm columns), so a single program serves all cores.
* Per edge: two scalar_tensor_tensor ops build the cubic argument from
  shared powers (w, w^2, b*w^3), one ACT Tanh (fp16 out), one fp16
  product with the source intensity, one PE matmul against ones
  (reduction over the k axis on partitions).  Per-slot src sums supply
  the saturated-edge values.  The host takes edge differences, scales,
  pools 4x4 spatially, and mirrors.
"""

import sys

for _p in ("/opt/trn_rl_repo",):
    if _p not in sys.path:
        sys.path.insert(0, _p)

import numpy as np

# ---------------- problem constants (compile-time, model-intrinsic) --------
IMAGE_RES = 128
VEL_RES = 80
VEL_UP = 5
IMG_UP = 4
N_CORES = 8
HALF_I = IMAGE_RES // 2            # 64 computed rows
NCOL = HALF_I * IMAGE_RES          # 8192 computed columns
FREE = 1024                        # free elements per core
NSLOT = 4
SLOTC = FREE // NSLOT              # 256 columns per slot
CUBE_FOV = 1000.0
M_TO_PC = 1.0 / 3.086e16
V_MAX_PC = np.float32(200000.0 * M_TO_PC)
R_C = np.float32(0.1 * CUBE_FOV)
R_D = np.float32(0.3 * CUBE_FOV)
H_Z = np.float32(0.05 * CUBE_FOV)
VEL_MIN = -300000.0
VEL_MAX = 300000.0

RC2 = float(R_C) * float(R_C)
INV_RD2 = 1.0 / (float(R_D) * float(R_D))
NEG_H = -1.0 / (2.0 * float(H_Z) * float(H_Z))
EPS_R2D2 = np.float32(1e-25)

# tanh(a*y + b*y^3) ~= erf(y), max err 2.8e-4
A_FIT = 1.12967583
B_FIT = 0.09979283
MARGIN_Y = 2.6          # |y| beyond which tanh-cubic is saturated (err 1.7e-4)
RANGE_PAD = 0.08        # host/device fp32 discrepancy pad on u ranges

# sm column layout
_C_NSZ = 0     # -si * z_k          (per partition)
_C_CIZ = 1     # ci * z_k           (per partition)
_C_CI = 2      # ci
_C_SI = 3      # si
_C_SC2 = 4     # scmag^2  (su scale)
_C_COEF = 8    # then 3 cols (B, C, D) per edge-slot

_CACHE = {}


# ====================== device program ======================

def _build_program(widths):
    from concourse import bacc, mybir, tile

    f32 = mybir.dt.float32
    f16 = mybir.dt.float16
    AF = mybir.ActivationFunctionType
    OP = mybir.AluOpType

    NW = sum(widths)                       # edge-slots (e.g. 25)
    NOUT = NW + NSLOT                      # + per-slot src sums
    NBANK = (NOUT + 5) // 6                # PSUM banks used (6 outs/bank:
    SM_COLS = _C_COEF + 3 * NW             #  3 rows {0,32,64} x 2 col halves)

    nc = bacc.Bacc(None)

    pk_d = nc.dram_tensor("pk", [128, 3 * FREE], f32, kind="ExternalInput")
    sm_d = nc.dram_tensor("sm", [128, SM_COLS], f32, kind="ExternalInput")
    ones_d = nc.dram_tensor("ones", [128, 32], f16, kind="ExternalInput")
    out_d = nc.dram_tensor("out", [3 * NBANK, 512], f32, kind="ExternalOutput")

    H = FREE // 2

    with tile.TileContext(nc) as tc:
        with (
            tc.tile_pool(name="inp", bufs=1) as inp,
            tc.tile_pool(name="fld", bufs=1) as fld,
            tc.tile_pool(name="argp", bufs=8) as argp,
            tc.tile_pool(name="ep", bufs=8) as ep,
            tc.tile_pool(name="pp", bufs=8) as pp,
            tc.tile_pool(name="psum", bufs=8, space="PSUM") as psum,
            tc.tile_pool(name="obp", bufs=5) as obp,
        ):
            pk = inp.tile([128, 3 * FREE], f32)
            sm = inp.tile([128, SM_COLS], f32)
            ones = inp.tile([128, 32], f16)
            warm = inp.tile([128, 1], f32)

            # small inputs on the gpsimd SWDGE queue; pk chunks on sync,
            # ordered so the h0 field chain unblocks first (P2 only needed
            # late, at w = su*P2)
            nc.gpsimd.dma_start(sm[:], sm_d[:])
            nc.gpsimd.dma_start(ones[:], ones_d[:])
            # pk layout: [y1 (0:1024) | P2 (1024:2048) | rx2eps (2048:3072)]
            for c0, c1 in ((0, H), (2 * FREE, 2 * FREE + H),
                           (H, FREE), (2 * FREE + H, 3 * FREE),
                           (FREE, FREE + H), (FREE + H, 2 * FREE)):
                nc.sync.dma_start(pk[:, c0:c1], pk_d[:, c0:c1])

            y1 = pk[:, 0:FREE]
            P2 = pk[:, FREE:2 * FREE]
            rx2 = pk[:, 2 * FREE:3 * FREE]

            def col(i):
                return sm[:, i:i + 1]

            V = nc.vector
            G = nc.gpsimd
            ACT = nc.scalar

            # field tiles (full width; halves computed on different engines)
            roty = fld.tile([128, FREE], f32)
            rotz = fld.tile([128, FREE], f32)
            y2 = fld.tile([128, FREE], f32)
            z2 = fld.tile([128, FREE], f32)
            r2d2 = fld.tile([128, FREE], f32)
            q = fld.tile([128, FREE], f32)
            den = fld.tile([128, FREE], f32)
            rec = fld.tile([128, FREE], f32)
            scr = fld.tile([128, FREE], f32)
            up = fld.tile([128, FREE], f32)
            su = fld.tile([128, FREE], f32)
            w = fld.tile([128, FREE], f32)
            W2 = fld.tile([128, FREE], f32)
            U3B = fld.tile([128, FREE], f32)
            slq = fld.tile([128, FREE], f32)
            sarg = fld.tile([128, FREE], f32)
            tmp = fld.tile([128, FREE], f32)
            src = fld.tile([128, FREE], f16)

            # ACT: warm the sqrt table while DMAs run (sm arrives first)
            ACT.activation(warm[:, :], col(_C_SC2), AF.Sqrt)

            # ---- field: half 0 on DVE, half 1 on Pool ----
            # Pool stt supports only (mult, add); decompose elsewhere.
            def field_half(E, s, is_dve):
                E.tensor_scalar(roty[:, s], y1[:, s], col(_C_CI), col(_C_NSZ),
                                op0=OP.mult, op1=OP.add)
                E.tensor_scalar(rotz[:, s], y1[:, s], col(_C_SI), col(_C_CIZ),
                                op0=OP.mult, op1=OP.add)
                E.tensor_mul(y2[:, s], roty[:, s], roty[:, s])
                E.tensor_mul(z2[:, s], rotz[:, s], rotz[:, s])
                E.tensor_add(r2d2[:, s], y2[:, s], rx2[:, s])
                # slq early: only needs r2d2 (emitted into ACT stream below)
                E.tensor_add(q[:, s], r2d2[:, s], z2[:, s])
                if is_dve:
                    E.scalar_tensor_tensor(den[:, s], q[:, s], RC2, r2d2[:, s],
                                           op0=OP.add, op1=OP.mult)
                else:
                    E.tensor_scalar_add(tmp[:, s], q[:, s], RC2)
                    E.tensor_mul(den[:, s], tmp[:, s], r2d2[:, s])

            def field_post(E, s, is_dve):
                # up = (q * scmag2) * rec  -> su = sqrt(up) on ACT
                if is_dve:
                    E.scalar_tensor_tensor(up[:, s], q[:, s], col(_C_SC2),
                                           rec[:, s], op0=OP.mult, op1=OP.mult)
                else:
                    E.tensor_scalar_mul(tmp[:, s], q[:, s], col(_C_SC2))
                    E.tensor_mul(up[:, s], tmp[:, s], rec[:, s])

            def field_w(E, s, is_dve):
                E.tensor_mul(w[:, s], su[:, s], P2[:, s])

            def field_tail(E, s, is_dve):
                # U3B = (W2 * b) * w ; sarg = (z2 * negH) - slq
                if is_dve:
                    E.scalar_tensor_tensor(U3B[:, s], W2[:, s], B_FIT, w[:, s],
                                           op0=OP.mult, op1=OP.mult)
                    E.scalar_tensor_tensor(sarg[:, s], z2[:, s], NEG_H,
                                           slq[:, s], op0=OP.mult,
                                           op1=OP.subtract)
                else:
                    E.tensor_scalar_mul(tmp[:, s], W2[:, s], B_FIT)
                    E.tensor_mul(U3B[:, s], tmp[:, s], w[:, s])
                    E.tensor_scalar_mul(tmp[:, s], z2[:, s], NEG_H)
                    E.tensor_sub(sarg[:, s], tmp[:, s], slq[:, s])

            s0 = slice(0, H)
            s1 = slice(H, FREE)
            field_half(V, s0, True)
            field_half(G, s1, False)
            # reciprocal is DVE-only
            V.reciprocal_approx_accurate(rec[:, s0], den[:, s0], scr[:, s0])
            V.reciprocal_approx_accurate(rec[:, s1], den[:, s1], scr[:, s1])
            field_post(V, s0, True)
            field_post(G, s1, False)

            # ACT sqrt phase (strict order: every Sqrt before any Exp/Tanh)
            ACT.activation(slq[:, s0], r2d2[:, s0], AF.Sqrt, scale=INV_RD2)
            ACT.activation(slq[:, s1], r2d2[:, s1], AF.Sqrt, scale=INV_RD2)
            ACT.activation(su[:, s0], up[:, s0], AF.Sqrt)
            field_w(V, s0, True)
            ACT.activation(su[:, s1], up[:, s1], AF.Sqrt)
            field_w(G, s1, False)
            ACT.activation(W2[:, s0], w[:, s0], AF.Square)
            ACT.activation(W2[:, s1], w[:, s1], AF.Square)

            field_tail(V, s0, True)
            field_tail(G, s1, False)

            # exp table from here on
            ACT.activation(src[:, s0], sarg[:, s0], AF.Exp)
            ACT.activation(src[:, s1], sarg[:, s1], AF.Exp)

            # ---- KDE edge evaluations ----
            bank_tiles = {}
            bank_fill = {}
            pend_out = []

            def psum_slot(k):
                b, pos = k // 6, k % 6
                if b not in bank_tiles:
                    bank_tiles[b] = psum.tile([128, 512], f32, tag="acc",
                                              name=f"bank{b}")
                    bank_fill[b] = 0
                row = 32 * (pos % 3)
                ch = pos // 3
                return b, bank_tiles[b][row:row + 32, 256 * ch:256 * (ch + 1)]

            def emit_mm(k, rhs):
                b, dst = psum_slot(k)
                nc.tensor.matmul(dst, ones[:, :], rhs, start=True, stop=True,
                                 skip_group_check=True)
                bank_fill[b] += 1
                if bank_fill[b] == 6:
                    # gpsimd cannot access PSUM; alternate DVE / ACT copies
                    ob = obp.tile([65, 512], f32, tag="ob", name=f"ob{b}")
                    if b % 2 == 0:
                        V.tensor_copy(ob[:, :], bank_tiles[b][0:65, :])
                    else:
                        ACT.activation(ob[:, :], bank_tiles[b][0:65, :],
                                       AF.Identity)
                    nc.sync.dma_start(out_d[3 * b:3 * b + 3, :],
                                      ob[0:65:32, :])

            # per-slot src sums (saturated-edge values)
            for s in range(NSLOT):
                emit_mm(s, src[:, SLOTC * s:SLOTC * (s + 1)])

            # greedy DVE/Pool balance for the per-edge arg/product ops
            # (ns estimates incl. seq overhead; gpsimd has no stt, so its
            # args are decomposed into 4 ops)
            eng_t = {"v": 0.0, "g": 0.0}
            ARG_V, ARG_G = 2 * 312.0, 4 * 249.0   # args [*,256]
            PRD_V, PRD_G = 178.0, 249.0           # fp16 product [*,256]

            k = NSLOT
            for s in range(NSLOT):
                cs = slice(SLOTC * s, SLOTC * (s + 1))
                for l in range(widths[s]):
                    cb = _C_COEF + 3 * (sum(widths[:s]) + l)
                    h = argp.tile([128, SLOTC], f32, tag="h")
                    ap = argp.tile([128, SLOTC], f32, tag="ap")
                    use_v = (eng_t["v"] + ARG_V <= eng_t["g"] + ARG_G)
                    eng_t["v" if use_v else "g"] += ARG_V if use_v else ARG_G
                    if use_v:
                        V.scalar_tensor_tensor(h[:, :], W2[:, cs], col(cb),
                                               U3B[:, cs], op0=OP.mult,
                                               op1=OP.add)
                        V.scalar_tensor_tensor(ap[:, :], w[:, cs],
                                               col(cb + 1), h[:, :],
                                               op0=OP.mult, op1=OP.add)
                    else:
                        ht = argp.tile([128, SLOTC], f32, tag="ht")
                        apt = argp.tile([128, SLOTC], f32, tag="apt")
                        G.tensor_scalar_mul(ht[:, :], W2[:, cs], col(cb))
                        G.tensor_add(h[:, :], ht[:, :], U3B[:, cs])
                        G.tensor_scalar_mul(apt[:, :], w[:, cs], col(cb + 1))
                        G.tensor_add(ap[:, :], apt[:, :], h[:, :])
                    e = ep.tile([128, SLOTC], f16, tag="e")
                    ACT.activation(e[:, :], ap[:, :], AF.Tanh,
                                   bias=col(cb + 2))
                    p = pp.tile([128, SLOTC], f16, tag="p")
                    use_v = (eng_t["v"] + PRD_V <= eng_t["g"] + PRD_G)
                    E2 = V if use_v else G
                    eng_t["v" if use_v else "g"] += PRD_V if use_v else PRD_G
                    E2.tensor_mul(p[:, :], src[:, cs], e[:, :])
                    emit_mm(k, p[:, :])
                    k += 1

            # pad the last bank so its copy reads only written PSUM
            while k % 6 != 0:
                emit_mm(k, src[:, 0:SLOTC])
                k += 1

    nc.finalize()
    return nc


# ====================== host side ======================

def _f32(x):
    return np.float32(x)


def _host_field(inc, rot, lb):
    """Host replica of the device field (f32) for window computation and
    input-plane construction."""
    f32 = np.float32
    ci, si = f32(np.cos(inc)), f32(np.sin(inc))
    cr, sr = f32(np.cos(rot)), f32(np.sin(rot))
    lin = np.linspace(-CUBE_FOV, CUBE_FOV, IMAGE_RES, dtype=f32)
    zl = np.linspace(f32(VEL_MIN * M_TO_PC), f32(VEL_MAX * M_TO_PC),
                     VEL_RES, dtype=f32)
    D = f32(zl[1] - zl[0])
    sig = f32(abs(lb))
    beta = f32(D / sig)

    x = lin[:HALF_I][:, None]
    y = lin[None, :]
    y1 = (sr * x + cr * y).astype(f32)                  # [64,128]
    rotx = (cr * x - sr * y).astype(f32)
    sgn = f32(-1.0) if si >= 0 else f32(1.0)
    P2 = (sgn * rotx).astype(f32)
    rx2 = (rotx * rotx + EPS_R2D2).astype(f32)
    scmag = f32(abs(si) * V_MAX_PC * beta / D)

    zk = lin
    roty = (ci * y1[:, :, None] + (-si * zk)[None, None, :]).astype(f32)
    rotz = (si * y1[:, :, None] + (ci * zk)[None, None, :]).astype(f32)
    y2 = roty * roty
    z2 = rotz * rotz
    r2d2 = (y2 + rx2[:, :, None]).astype(f32)
    q = (r2d2 + z2).astype(f32)
    den = ((q + f32(RC2)) * r2d2).astype(f32)
    rec = (f32(1.0) / den).astype(f32)
    up = (q * f32(scmag * scmag) * rec).astype(f32)
    su = np.sqrt(up).astype(f32)
    w = (su * P2[:, :, None]).astype(f32)               # beta * vz / D
    u = (w / beta + f32((VEL_RES / 2) - 0.5)).astype(f32)
    return dict(y1=y1, P2=P2, rx2=rx2, u=u, beta=beta, D=D, sig=sig,
                ci=ci, si=si, zk=zk, scmag=scmag)


def _prep(inc, rot, lb):
    f32 = np.float32
    F = _host_field(inc, rot, lb)
    beta = float(F["beta"])

    # per-column active edge intervals
    Mu = MARGIN_Y / beta + RANGE_PAD
    col_lo = F["u"].min(axis=2).ravel() - RANGE_PAD
    col_hi = F["u"].max(axis=2).ravel() + RANGE_PAD
    t0 = np.maximum(np.ceil((col_lo - Mu + 0.5) / 5).astype(int), 0)
    t1 = np.minimum(np.floor((col_hi + Mu + 0.5) / 5).astype(int), 16)
    t1 = np.maximum(t1, t0)

    order = np.lexsort((t1, t0))
    ngroups = N_CORES * NSLOT
    groups = [order[SLOTC * g:SLOTC * (g + 1)] for g in range(ngroups)]
    g_t0 = np.array([t0[g].min() for g in groups])
    g_t1 = np.array([t1[g].max() for g in groups])
    g_w = g_t1 - g_t0 + 1

    # slot classes: 8 widest -> slot 0, next 8 -> slot 1, ...
    gorder = np.argsort(-g_w, kind="stable")
    widths = []
    slot_groups = []            # slot_groups[s][core] = group idx
    for s in range(NSLOT):
        cls = gorder[N_CORES * s:N_CORES * (s + 1)]
        widths.append(int(g_w[cls].max()))
        slot_groups.append(list(cls))
    widths = tuple(widths)
    NW = sum(widths)

    # per (core, slot): window start T0 (covers group, stays in [0,16])
    T0 = np.zeros((N_CORES, NSLOT), dtype=int)
    cols = np.zeros((N_CORES, NSLOT, SLOTC), dtype=int)
    for s in range(NSLOT):
        for c in range(N_CORES):
            g = slot_groups[s][c]
            T0[c, s] = max(0, min(int(g_t0[g]), 17 - widths[s]))
            cols[c, s] = groups[g]

    # coefficient tables: per edge t, y = w - c_t with c_t = beta*(5t - 40);
    # arg = b*w^3 - 3*b*c*w^2 + (a + 3*b*c^2)*w + (-a*c - b*c^3)
    a, b = A_FIT, B_FIT
    SM_COLS = _C_COEF + 3 * NW
    sm_all = []
    pk_all = []
    for c in range(N_CORES):
        sm = np.zeros((128, SM_COLS), dtype=f32)
        sm[:, _C_NSZ] = (-F["si"] * F["zk"]).astype(f32)
        sm[:, _C_CIZ] = (F["ci"] * F["zk"]).astype(f32)
        sm[:, _C_CI] = F["ci"]
        sm[:, _C_SI] = F["si"]
        sm[:, _C_SC2] = f32(F["scmag"] * F["scmag"])
        off = 0
        for s in range(NSLOT):
            for l in range(widths[s]):
                t = T0[c, s] + l
                ct = beta * (5.0 * t - (VEL_RES / 2.0))
                base = _C_COEF + 3 * (off + l)
                sm[:, base + 0] = f32(-3.0 * b * ct)
                sm[:, base + 1] = f32(a + 3.0 * b * ct * ct)
                sm[:, base + 2] = f32(-a * ct - b * ct ** 3)
            off += widths[s]
        ccols = cols[c].reshape(-1)
        pkrow = np.concatenate([
            F["y1"].ravel()[ccols],
            F["P2"].ravel()[ccols],
            F["rx2"].ravel()[ccols],
        ]).astype(f32)
        pk = np.ascontiguousarray(np.broadcast_to(pkrow, (128, 3 * FREE)))
        sm_all.append(sm)
        pk_all.append(pk)

    ones = np.ones((128, 32), dtype=np.float16)
    in_maps = [{"pk": pk_all[c], "sm": sm_all[c], "ones": ones}
               for c in range(N_CORES)]
    meta = dict(widths=widths, T0=T0, cols=cols, beta=beta,
                sig=float(F["sig"]))
    return in_maps, meta


def _assemble(results, meta):
    f64 = np.float64
    widths = meta["widths"]
    T0 = meta["T0"]
    cols = meta["cols"]
    beta = meta["beta"]
    sig = meta["sig"]
    NW = sum(widths)
    NOUT = NW + NSLOT

    half = np.zeros((16, NCOL), dtype=f64)
    for c in range(N_CORES):
        out = np.asarray(results[c]["out"], dtype=f64)   # [3*NBANK, 512]

        def vec(k):
            b, pos = k // 6, k % 6
            return out[3 * b + (pos % 3), 256 * (pos // 3):
                       256 * (pos // 3) + 256]

        off = 0
        for s in range(NSLOT):
            S = vec(s)
            Eb = np.zeros((18, SLOTC), dtype=f64)
            lo = T0[c, s]
            hi = lo + widths[s] - 1
            for t in range(17):
                if t < lo:
                    Eb[t] = S
                elif t > hi:
                    Eb[t] = -S
                else:
                    Eb[t] = vec(NSLOT + off + (t - lo))
            cube = Eb[0:16] - Eb[1:17]                  # [16, SLOTC]
            half[:, cols[c, s]] = cube
            off += widths[s]

    scale = np.sqrt(np.pi) / (2.0 * beta)
    pref = 1.0 / np.sqrt(2.0 * np.pi * sig * sig)
    half = half.reshape(16, HALF_I, IMAGE_RES) * (scale * pref / VEL_UP)
    halfp = half.reshape(16, 16, 4, 32, 4).mean(axis=(2, 4))
    full = np.empty((16, 32, 32), dtype=np.float64)
    full[:, :16, :] = halfp
    full[:, 16:, :] = halfp[::-1, ::-1, ::-1]
    return full.astype(np.float32)


def _get_prog(widths):
    key = ("nc", widths)
    if key not in _CACHE:
        _CACHE[key] = _build_program(widths)
    _CACHE["nc"] = _CACHE[key]
    return _CACHE[key]


def _host_inputs(inclination, sky_rot, line_broadening):
    key = ("prep", float(inclination), float(sky_rot), float(line_broadening))
    if key not in _CACHE:
        _CACHE[key] = _prep(float(inclination), float(sky_rot),
                            float(line_broadening))
    in_maps, meta = _CACHE[key]
    _get_prog(meta["widths"])
    _CACHE["meta"] = meta
    return in_maps


def _run(in_maps, trace=False, **kwargs):
    from concourse.bass_utils import run_bass_kernel_spmd
    nc = _CACHE["nc"]
    return run_bass_kernel_spmd(nc, in_maps, list(range(N_CORES)),
                                trace=trace, **kwargs)


def kernel(inclination, sky_rot, line_broadening):
    in_maps = _host_inputs(inclination, sky_rot, line_broadening)
    res = _run(in_maps)
    return _assemble(res.results, _CACHE["meta"])


# revision 13
# speedup vs baseline: 3.1279x; 1.0558x over previous
"""Trainium2 Bass kernel for nn_CubeSimulator.

Reference computation: a 128^3 spatial grid is rotated (Rz(sky_rot) then
Rx(inclination)), a rotation-curve velocity field and an exponential-disk
intensity field are evaluated, an 80-channel Gaussian KDE over the
line-of-sight velocity reduces the third grid axis, and the [80,128,128]
cube is avg-pooled (5,4,4) to [16,32,32].

Kernel strategy
---------------
* Point-reflection symmetry (as in the earlier exp-based kernel): only the
  i < 64 half of the sky plane is computed; the host mirrors the pooled
  output (cube[vo, 127-i, 127-j] == cube[15-vo, i, j]).
* erf-difference identity: the output only needs GROUPS of 5 adjacent
  velocity channels summed (the avg-pool).  The channel spacing is
  0.38 sigma, so the sum of the 5 Gaussians over a group equals
  (sqrt(pi)/2beta) * [erf(beta(u-5t+.5)) - erf(beta(u-5(t+1)+.5))]
  to ~0.6% of the group max (Euler-Maclaurin edge error), where
  u = (vz - z_0)/Delta is the velocity in channel units and t indexes
  GROUP EDGES.  Adjacent groups share edges, so 17 edge evaluations
  replace 80 exponentials - and the saturated edges (|arg| large for
  every point of a shard) are free (+-1 exactly).
* erf itself is evaluated as tanh(a*y + b*y^3) (max fit error 2.8e-4);
  tanh shares the ACT 'exp_and_others' table with exp, so the whole
  program needs only sqrt -> exp_and_others, one table switch.
* Column sharding: the 8192 computed (i,j) columns are sorted by their
  active-edge interval and packed into 32 groups of 256 columns; each
  core gets 4 slot-groups with per-slot edge windows (SPMD instruction
  count = sum of per-slot-class maximum widths, ~25 edge-slots of 256
  columns vs 13 full-width edges).  Edge positions/coefficients are
  per-core DATA (sm columns), so a single program serves all cores.
* Per edge: two scalar_tensor_tensor ops build the cubic argument from
  shared powers (w, w^2, b*w^3), one ACT Tanh (fp16 out), one fp16
  product with the source intensity, one PE matmul against ones
  (reduction over the k axis on partitions).  Per-slot src sums supply
  the saturated-edge values.  The host takes edge differences, scales,
  pools 4x4 spatially, and mirrors.
"""

import sys

for _p in ("/opt/trn_rl_repo",):
    if _p not in sys.path:
        sys.path.insert(0, _p)

import numpy as np

# ---------------- problem constants (compile-time, model-intrinsic) --------
IMAGE_RES = 128
VEL_RES = 80
VEL_UP = 5
IMG_UP = 4
N_CORES = 8
HALF_I = IMAGE_RES // 2            # 64 computed rows
NCOL = HALF_I * IMAGE_RES          # 8192 computed columns
FREE = 1024                        # free elements per core
NSLOT = 4
SLOTC = FREE // NSLOT              # 256 columns per slot
CUBE_FOV = 1000.0
M_TO_PC = 1.0 / 3.086e16
V_MAX_PC = np.float32(200000.0 * M_TO_PC)
R_C = np.float32(0.1 * CUBE_FOV)
R_D = np.float32(0.3 * CUBE_FOV)
H_Z = np.float32(0.05 * CUBE_FOV)
VEL_MIN = -300000.0
VEL_MAX = 300000.0

RC2 = float(R_C) * float(R_C)
INV_RD2 = 1.0 / (float(R_D) * float(R_D))
NEG_H = -1.0 / (2.0 * float(H_Z) * float(H_Z))
EPS_R2D2 = np.float32(1e-25)

# tanh(a*y + b*y^3) ~= erf(y), max err 2.8e-4
A_FIT = 1.12967583
B_FIT = 0.09979283
MARGIN_Y = 2.6          # |y| beyond which tanh-cubic is saturated (err 1.7e-4)
RANGE_PAD = 0.08        # host/device fp32 discrepancy pad on u ranges

# sm column layout
_C_NSZ = 0     # -si * z_k          (per partition)
_C_CIZ = 1     # ci * z_k           (per partition)
_C_CI = 2      # ci
_C_SI = 3      # si
_C_SC2 = 4     # scmag^2  (su scale)
_C_COEF = 8    # then 3 cols (B, C, D) per edge-slot

_CACHE = {}


# ====================== device program ======================

def _build_program(widths):
    from concourse import bacc, mybir, tile

    f32 = mybir.dt.float32
    f16 = mybir.dt.float16
    AF = mybir.ActivationFunctionType
    OP = mybir.AluOpType

    NW = sum(widths)                       # edge-slots (e.g. 25)
    NOUT = NW + NSLOT                      # + per-slot src sums
    NBANK = (NOUT + 5) // 6                # PSUM banks used (6 outs/bank:
    SM_COLS = _C_COEF + 3 * NW             #  3 rows {0,32,64} x 2 col halves)

    nc = bacc.Bacc(None)

    pk_d = nc.dram_tensor("pk", [128, 3 * FREE], f32, kind="ExternalInput")
    sm_d = nc.dram_tensor("sm", [128, SM_COLS], f32, kind="ExternalInput")
    ones_d = nc.dram_tensor("ones", [128, 32], f16, kind="ExternalInput")
    out_d = nc.dram_tensor("out", [3 * NBANK, 512], f32, kind="ExternalOutput")

    H = FREE // 2

    with tile.TileContext(nc) as tc:
        with (
            tc.tile_pool(name="inp", bufs=1) as inp,
            tc.tile_pool(name="fld", bufs=1) as fld,
            tc.tile_pool(name="argp", bufs=8) as argp,
            tc.tile_pool(name="ep", bufs=8) as ep,
            tc.tile_pool(name="pp", bufs=8) as pp,
            tc.tile_pool(name="psum", bufs=8, space="PSUM") as psum,
            tc.tile_pool(name="obp", bufs=5) as obp,
        ):
            pk = inp.tile([128, 3 * FREE], f32)
            sm = inp.tile([128, SM_COLS], f32)
            ones = inp.tile([128, 32], f16)
            warm = inp.tile([128, 1], f32)

            # small inputs on the gpsimd SWDGE queue; pk chunks on sync,
            # ordered so the h0 field chain unblocks first (P2 only needed
            # late, at w = su*P2)
            nc.gpsimd.dma_start(sm[:], sm_d[:])
            nc.gpsimd.dma_start(ones[:], ones_d[:])
            # pk layout: [y1 (0:1024) | P2 (1024:2048) | rx2eps (2048:3072)]
            for c0, c1 in ((0, H), (2 * FREE, 2 * FREE + H),
                           (H, FREE), (2 * FREE + H, 3 * FREE),
                           (FREE, FREE + H), (FREE + H, 2 * FREE)):
                nc.sync.dma_start(pk[:, c0:c1], pk_d[:, c0:c1])

            y1 = pk[:, 0:FREE]
            P2 = pk[:, FREE:2 * FREE]
            rx2 = pk[:, 2 * FREE:3 * FREE]

            def col(i):
                return sm[:, i:i + 1]

            V = nc.vector
            G = nc.gpsimd
            ACT = nc.scalar

            # field tiles (full width; halves computed on different engines)
            roty = fld.tile([128, FREE], f32)
            rotz = fld.tile([128, FREE], f32)
            y2 = fld.tile([128, FREE], f32)
            z2 = fld.tile([128, FREE], f32)
            r2d2 = fld.tile([128, FREE], f32)
            q = fld.tile([128, FREE], f32)
            den = fld.tile([128, FREE], f32)
            rec = fld.tile([128, FREE], f32)
            up = fld.tile([128, FREE], f32)
            su = fld.tile([128, FREE], f32)
            w = fld.tile([128, FREE], f32)
            W2 = fld.tile([128, FREE], f32)
            U3B = fld.tile([128, FREE], f32)
            slq = fld.tile([128, FREE], f32)
            sarg = fld.tile([128, FREE], f32)
            tmp = fld.tile([128, FREE], f32)
            src = fld.tile([128, FREE], f16)

            # ACT: warm the sqrt table while DMAs run (sm arrives first)
            ACT.activation(warm[:, :], col(_C_SC2), AF.Sqrt)

            # ---- field: half 0 on DVE, half 1 on Pool ----
            # Pool stt supports only (mult, add); decompose elsewhere.
            def field_half(E, s, is_dve):
                E.tensor_scalar(roty[:, s], y1[:, s], col(_C_CI), col(_C_NSZ),
                                op0=OP.mult, op1=OP.add)
                E.tensor_scalar(rotz[:, s], y1[:, s], col(_C_SI), col(_C_CIZ),
                                op0=OP.mult, op1=OP.add)
                E.tensor_mul(y2[:, s], roty[:, s], roty[:, s])
                E.tensor_mul(z2[:, s], rotz[:, s], rotz[:, s])
                E.tensor_add(r2d2[:, s], y2[:, s], rx2[:, s])
                # slq early: only needs r2d2 (emitted into ACT stream below)
                E.tensor_add(q[:, s], r2d2[:, s], z2[:, s])
                if is_dve:
                    E.scalar_tensor_tensor(den[:, s], q[:, s], RC2, r2d2[:, s],
                                           op0=OP.add, op1=OP.mult)
                else:
                    E.tensor_scalar_add(tmp[:, s], q[:, s], RC2)
                    E.tensor_mul(den[:, s], tmp[:, s], r2d2[:, s])

            def field_post(E, s, is_dve):
                # su = sqrt(up * scmag2): scale folded into the ACT Sqrt
                E.tensor_mul(up[:, s], q[:, s], rec[:, s])

            def field_w(E, s, is_dve):
                E.tensor_mul(w[:, s], su[:, s], P2[:, s])

            def field_tail(E, s, is_dve):
                # U3B = W2b * w  (W2b = b*w^2, sqrt(b) folded into Square)
                # sarg = z2 + slq'  with slq' = sqrt(r2d2)/(RD*|negH|);
                # src = exp(negH * sarg) (scale folded into the ACT Exp)
                E.tensor_mul(U3B[:, s], W2[:, s], w[:, s])
                E.tensor_add(sarg[:, s], z2[:, s], slq[:, s])

            s0 = slice(0, H)
            s1 = slice(H, FREE)
            field_half(V, s0, True)
            field_half(G, s1, False)
            # reciprocal is DVE-only
            V.reciprocal_approx_fast(rec[:, s0], den[:, s0])
            V.reciprocal_approx_fast(rec[:, s1], den[:, s1])
            field_post(V, s0, True)
            field_post(G, s1, False)

            # ACT sqrt phase (strict order: every Sqrt before any Exp/Tanh)
            SLQSC = INV_RD2 / (NEG_H * NEG_H)
            ACT.activation(slq[:, s0], r2d2[:, s0], AF.Sqrt, scale=SLQSC)
            ACT.activation(slq[:, s1], r2d2[:, s1], AF.Sqrt, scale=SLQSC)
            ACT.activation(su[:, s0], up[:, s0], AF.Sqrt, scale=col(_C_SC2))
            field_w(V, s0, True)
            ACT.activation(su[:, s1], up[:, s1], AF.Sqrt, scale=col(_C_SC2))
            field_w(G, s1, False)
            SQB = float(np.sqrt(B_FIT))
            ACT.activation(W2[:, s0], w[:, s0], AF.Square, scale=SQB)
            ACT.activation(W2[:, s1], w[:, s1], AF.Square, scale=SQB)

            field_tail(V, s0, True)
            field_tail(G, s1, False)

            # exp table from here on; the 1.0 scale column depends on both
            # su halves so the scheduler cannot hoist Exp between the Sqrts
            dcol = inp.tile([128, 2], f32)
            V.tensor_scalar_mul(dcol[:, 0:1], su[:, 0:1], 0.0)
            V.tensor_scalar(dcol[:, 1:2], dcol[:, 0:1], su[:, H:H + 1], NEG_H,
                            op0=OP.mult, op1=OP.add)
            ACT.activation(src[:, s0], sarg[:, s0], AF.Exp, scale=dcol[:, 1:2],
                           bias=0.0)
            ACT.activation(src[:, s1], sarg[:, s1], AF.Exp, scale=dcol[:, 1:2],
                           bias=0.0)

            # ---- KDE edge evaluations ----
            bank_tiles = {}
            bank_fill = {}
            pend_out = []

            def psum_slot(k):
                b, pos = k // 6, k % 6
                if b not in bank_tiles:
                    bank_tiles[b] = psum.tile([128, 512], f32, tag="acc",
                                              name=f"bank{b}")
                    bank_fill[b] = 0
                row = 32 * (pos % 3)
                ch = pos // 3
                return b, bank_tiles[b][row:row + 32, 256 * ch:256 * (ch + 1)]

            def emit_mm(k, rhs):
                b, dst = psum_slot(k)
                nc.tensor.matmul(dst, ones[:, :], rhs, start=True, stop=True,
                                 skip_group_check=True)
                bank_fill[b] += 1
                if bank_fill[b] == 6:
                    # gpsimd cannot access PSUM; alternate DVE / ACT copies
                    ob = obp.tile([65, 512], f32, tag="ob", name=f"ob{b}")
                    if b % 2 == 0:
                        V.tensor_copy(ob[:, :], bank_tiles[b][0:65, :])
                    else:
                        ACT.activation(ob[:, :], bank_tiles[b][0:65, :],
                                       AF.Identity)
                    nc.sync.dma_start(out_d[3 * b:3 * b + 3, :],
                                      ob[0:65:32, :])

            # per-slot src sums (saturated-edge values)
            for s in range(NSLOT):
                emit_mm(s, src[:, SLOTC * s:SLOTC * (s + 1)])

            # greedy DVE/Pool balance for the per-edge arg/product ops
            # (ns estimates incl. seq overhead; gpsimd has no stt, so its
            # args are decomposed into 4 ops)
            eng_t = {"v": 0.0, "g": 0.0}
            ARG_V, ARG_G = 2 * 312.0, 4 * 249.0   # args [*,256]
            PRD_V, PRD_G = 178.0, 249.0           # fp16 product [*,256]

            k = NSLOT
            for s in range(NSLOT):
                cs = slice(SLOTC * s, SLOTC * (s + 1))
                for l in range(widths[s]):
                    cb = _C_COEF + 3 * (sum(widths[:s]) + l)
                    h = argp.tile([128, SLOTC], f32, tag="h")
                    ap = argp.tile([128, SLOTC], f32, tag="ap")
                    use_v = (eng_t["v"] + ARG_V <= eng_t["g"] + ARG_G)
                    eng_t["v" if use_v else "g"] += ARG_V if use_v else ARG_G
                    if use_v:
                        V.scalar_tensor_tensor(h[:, :], W2[:, cs], col(cb),
                                               U3B[:, cs], op0=OP.mult,
                                               op1=OP.add)
                        V.scalar_tensor_tensor(ap[:, :], w[:, cs],
                                               col(cb + 1), h[:, :],
                                               op0=OP.mult, op1=OP.add)
                    else:
                        ht = argp.tile([128, SLOTC], f32, tag="ht")
                        apt = argp.tile([128, SLOTC], f32, tag="apt")
                        G.tensor_scalar_mul(ht[:, :], W2[:, cs], col(cb))
                        G.tensor_add(h[:, :], ht[:, :], U3B[:, cs])
                        G.tensor_scalar_mul(apt[:, :], w[:, cs], col(cb + 1))
                        G.tensor_add(ap[:, :], apt[:, :], h[:, :])
                    e = ep.tile([128, SLOTC], f16, tag="e")
                    ACT.activation(e[:, :], ap[:, :], AF.Tanh,
                                   bias=col(cb + 2))
                    p = pp.tile([128, SLOTC], f16, tag="p")
                    use_v = (eng_t["v"] + PRD_V <= eng_t["g"] + PRD_G)
                    E2 = V if use_v else G
                    eng_t["v" if use_v else "g"] += PRD_V if use_v else PRD_G
                    E2.tensor_mul(p[:, :], src[:, cs], e[:, :])
                    emit_mm(k, p[:, :])
                    k += 1

            # pad the last bank so its copy reads only written PSUM
            while k % 6 != 0:
                emit_mm(k, src[:, 0:SLOTC])
                k += 1

    nc.finalize()
    return nc


# ====================== host side ======================

def _f32(x):
    return np.float32(x)


def _host_field(inc, rot, lb):
    """Host replica of the device field (f32) for window computation and
    input-plane construction."""
    f32 = np.float32
    ci, si = f32(np.cos(inc)), f32(np.sin(inc))
    cr, sr = f32(np.cos(rot)), f32(np.sin(rot))
    lin = np.linspace(-CUBE_FOV, CUBE_FOV, IMAGE_RES, dtype=f32)
    zl = np.linspace(f32(VEL_MIN * M_TO_PC), f32(VEL_MAX * M_TO_PC),
                     VEL_RES, dtype=f32)
    D = f32(zl[1] - zl[0])
    sig = f32(abs(lb))
    beta = f32(D / sig)

    x = lin[:HALF_I][:, None]
    y = lin[None, :]
    y1 = (sr * x + cr * y).astype(f32)                  # [64,128]
    rotx = (cr * x - sr * y).astype(f32)
    sgn = f32(-1.0) if si >= 0 else f32(1.0)
    P2 = (sgn * rotx).astype(f32)
    rx2 = (rotx * rotx + EPS_R2D2).astype(f32)
    scmag = f32(abs(si) * V_MAX_PC * beta / D)

    zk = lin
    roty = (ci * y1[:, :, None] + (-si * zk)[None, None, :]).astype(f32)
    rotz = (si * y1[:, :, None] + (ci * zk)[None, None, :]).astype(f32)
    y2 = roty * roty
    z2 = rotz * rotz
    r2d2 = (y2 + rx2[:, :, None]).astype(f32)
    q = (r2d2 + z2).astype(f32)
    den = ((q + f32(RC2)) * r2d2).astype(f32)
    rec = (f32(1.0) / den).astype(f32)
    up = (q * f32(scmag * scmag) * rec).astype(f32)
    su = np.sqrt(up).astype(f32)
    w = (su * P2[:, :, None]).astype(f32)               # beta * vz / D
    u = (w / beta + f32((VEL_RES / 2) - 0.5)).astype(f32)
    return dict(y1=y1, P2=P2, rx2=rx2, u=u, beta=beta, D=D, sig=sig,
                ci=ci, si=si, zk=zk, scmag=scmag)


def _prep(inc, rot, lb):
    f32 = np.float32
    F = _host_field(inc, rot, lb)
    beta = float(F["beta"])

    # per-column active edge intervals
    Mu = MARGIN_Y / beta + RANGE_PAD
    col_lo = F["u"].min(axis=2).ravel() - RANGE_PAD
    col_hi = F["u"].max(axis=2).ravel() + RANGE_PAD
    t0 = np.maximum(np.ceil((col_lo - Mu + 0.5) / 5).astype(int), 0)
    t1 = np.minimum(np.floor((col_hi + Mu + 0.5) / 5).astype(int), 16)
    t1 = np.maximum(t1, t0)

    order = np.lexsort((t1, t0))
    ngroups = N_CORES * NSLOT
    groups = [order[SLOTC * g:SLOTC * (g + 1)] for g in range(ngroups)]
    g_t0 = np.array([t0[g].min() for g in groups])
    g_t1 = np.array([t1[g].max() for g in groups])
    g_w = g_t1 - g_t0 + 1

    # slot classes: 8 widest -> slot 0, next 8 -> slot 1, ...
    gorder = np.argsort(-g_w, kind="stable")
    widths = []
    slot_groups = []            # slot_groups[s][core] = group idx
    for s in range(NSLOT):
        cls = gorder[N_CORES * s:N_CORES * (s + 1)]
        widths.append(int(g_w[cls].max()))
        slot_groups.append(list(cls))
    widths = tuple(widths)
    NW = sum(widths)

    # per (core, slot): window start T0 (covers group, stays in [0,16])
    T0 = np.zeros((N_CORES, NSLOT), dtype=int)
    cols = np.zeros((N_CORES, NSLOT, SLOTC), dtype=int)
    for s in range(NSLOT):
        for c in range(N_CORES):
            g = slot_groups[s][c]
            T0[c, s] = max(0, min(int(g_t0[g]), 17 - widths[s]))
            cols[c, s] = groups[g]

    # coefficient tables: per edge t, y = w - c_t with c_t = beta*(5t - 40);
    # arg = b*w^3 - 3*b*c*w^2 + (a + 3*b*c^2)*w + (-a*c - b*c^3)
    a, b = A_FIT, B_FIT
    SM_COLS = _C_COEF + 3 * NW
    sm_all = []
    pk_all = []
    for c in range(N_CORES):
        sm = np.zeros((128, SM_COLS), dtype=f32)
        sm[:, _C_NSZ] = (-F["si"] * F["zk"]).astype(f32)
        sm[:, _C_CIZ] = (F["ci"] * F["zk"]).astype(f32)
        sm[:, _C_CI] = F["ci"]
        sm[:, _C_SI] = F["si"]
        sm[:, _C_SC2] = f32(F["scmag"] * F["scmag"])
        off = 0
        for s in range(NSLOT):
            for l in range(widths[s]):
                t = T0[c, s] + l
                ct = beta * (5.0 * t - (VEL_RES / 2.0))
                base = _C_COEF + 3 * (off + l)
                sm[:, base + 0] = f32(-3.0 * ct)
                sm[:, base + 1] = f32(a + 3.0 * b * ct * ct)
                sm[:, base + 2] = f32(-a * ct - b * ct ** 3)
            off += widths[s]
        ccols = cols[c].reshape(-1)
        pkrow = np.concatenate([
            F["y1"].ravel()[ccols],
            F["P2"].ravel()[ccols],
            F["rx2"].ravel()[ccols],
        ]).astype(f32)
        pk = np.ascontiguousarray(np.broadcast_to(pkrow, (128, 3 * FREE)))
        sm_all.append(sm)
        pk_all.append(pk)

    ones = np.ones((128, 32), dtype=np.float16)
    in_maps = [{"pk": pk_all[c], "sm": sm_all[c], "ones": ones}
               for c in range(N_CORES)]
    meta = dict(widths=widths, T0=T0, cols=cols, beta=beta,
                sig=float(F["sig"]))
    return in_maps, meta


def _assemble(results, meta):
    f64 = np.float64
    widths = meta["widths"]
    T0 = meta["T0"]
    cols = meta["cols"]
    beta = meta["beta"]
    sig = meta["sig"]
    NW = sum(widths)
    NOUT = NW + NSLOT

    half = np.zeros((16, NCOL), dtype=f64)
    for c in range(N_CORES):
        out = np.asarray(results[c]["out"], dtype=f64)   # [3*NBANK, 512]

        def vec(k):
            b, pos = k // 6, k % 6
            return out[3 * b + (pos % 3), 256 * (pos // 3):
                       256 * (pos // 3) + 256]

        off = 0
        for s in range(NSLOT):
            S = vec(s)
            Eb = np.zeros((18, SLOTC), dtype=f64)
            lo = T0[c, s]
            hi = lo + widths[s] - 1
            for t in range(17):
                if t < lo:
                    Eb[t] = S
                elif t > hi:
                    Eb[t] = -S
                else:
                    Eb[t] = vec(NSLOT + off + (t - lo))
            cube = Eb[0:16] - Eb[1:17]                  # [16, SLOTC]
            half[:, cols[c, s]] = cube
            off += widths[s]

    scale = np.sqrt(np.pi) / (2.0 * beta)
    pref = 1.0 / np.sqrt(2.0 * np.pi * sig * sig)
    half = half.reshape(16, HALF_I, IMAGE_RES) * (scale * pref / VEL_UP)
    halfp = half.reshape(16, 16, 4, 32, 4).mean(axis=(2, 4))
    full = np.empty((16, 32, 32), dtype=np.float64)
    full[:, :16, :] = halfp
    full[:, 16:, :] = halfp[::-1, ::-1, ::-1]
    return full.astype(np.float32)


def _get_prog(widths):
    key = ("nc", widths)
    if key not in _CACHE:
        _CACHE[key] = _build_program(widths)
    _CACHE["nc"] = _CACHE[key]
    return _CACHE[key]


def _host_inputs(inclination, sky_rot, line_broadening):
    key = ("prep", float(inclination), float(sky_rot), float(line_broadening))
    if key not in _CACHE:
        _CACHE[key] = _prep(float(inclination), float(sky_rot),
                            float(line_broadening))
    in_maps, meta = _CACHE[key]
    _get_prog(meta["widths"])
    _CACHE["meta"] = meta
    return in_maps


def _run(in_maps, trace=False, **kwargs):
    from concourse.bass_utils import run_bass_kernel_spmd
    nc = _CACHE["nc"]
    return run_bass_kernel_spmd(nc, in_maps, list(range(N_CORES)),
                                trace=trace, **kwargs)


def kernel(inclination, sky_rot, line_broadening):
    in_maps = _host_inputs(inclination, sky_rot, line_broadening)
    res = _run(in_maps)
    return _assemble(res.results, _CACHE["meta"])
